# revision 20
# baseline (speedup 1.0000x reference)
"""MultiHeadSelfAttention2D Trainium2 kernel.

Full inputs -> shard batch (B=8) across 8 NeuronCores (1 image per core) ->
bass/Tile flash-attention-style kernel per core -> gather.

Per-core dataflow (feature-major, C=128 partitions, N=4096 tokens):
  Qf = (s*Wq) @ x + s*bq     (128 x N)   s = 1/sqrt(head_dim), folded on host
  Kf = Wk @ x + bk           (128 x N)
  Vaug = token-major V per (key-block, head): [V_h(32) | 1] so the PV matmul
         also accumulates the softmax denominator Z in psum row 32.
  Unit order is HEAD-major: for h in 4: for k in 32 key-blocks:
      S    = Kf_h[k].T @ Qf_h[qblk]      (PE, psum ring slot, 128x512)
      P    = exp(S)                      (split across ACT engine [exact Exp]
                                          and DVE [Schraudolph bit-trick exp:
                                          bf16бит = round(A*x+B) as int16])
      O_h += Vaug_hk . P                 (PE, accumulated over k into ot bank)
  After each head's last key-block (so the normalize pipeline of head h
  overlaps the attention of head h+1):
      Z_h row -> SBUF via DMA; 1/Z via 2-op approx reciprocal (DVE);
      partition-broadcast of 1/Z via tiny PE matmul (ones[1x32] outer);
      ACT copies broadcast psum->SBUF; DVE multiplies O_h * (1/Z) -> of_h.
  Per qblk tail: out += Wo_h.T @ of_h (4 accumulating f32r matmuls) + bias.

All tail work is emitted lazily with per-step due-units so the in-order PE
queue never blocks on the normalize chain latency.
"""

import numpy as np

EMBED = 128
HEADS = 4
HD = 32
P = 128

_CACHE = {}

# This container's walrus build only accepts one sync-wait per Drain
# instruction; Tile's tail drain carries one wait per live proc. Split the
# extra waits onto standalone EventSemaphore instructions (same engine,
# earlier in program order - semantically identical).
_DRAIN_MAX_WAITS = 1


def _split_multiwait_drains(nc, mybir, bass_rust):
    for fn in nc.m.functions:
        for bb in fn.blocks:
            new = []
            changed = False
            for inst in bb.instructions:
                si = inst.sync_info
                if (si is not None
                        and len(si.on_wait) > _DRAIN_MAX_WAITS):
                    changed = True
                    waits = list(si.on_wait)
                    for j, w in enumerate(waits[_DRAIN_MAX_WAITS:]):
                        es = mybir.InstEventSemaphore(
                            name=f"{inst.name}-wsplit{j}", ins=[], outs=[])
                        es.engine = inst.engine
                        es.sync_info = bass_rust.SyncInfo(
                            on_wait=[w], on_update=[])
                        nc.register_instruction(es)
                        new.append(es)
                    inst.sync_info = bass_rust.SyncInfo(
                        on_wait=waits[:_DRAIN_MAX_WAITS],
                        on_update=list(si.on_update))
                new.append(inst)
            if changed:
                bb.instructions = new


# Schraudolph exp in bf16 bit space: bf16bits(exp(x)) ~ round(A*x + B).
# Two evaluations half a sawtooth period apart (B and B-64) summed with a
# plain ADD cancel most of the linearization sawtooth (+-0.5% vs +-3%): the
# bit-space -64 shift is simultaneously the half-period phase shift AND the
# 2^-1/2 mixture weight. The -105.25 offset folds the constant factor so
# ACT-exp blocks and DVE-exp blocks agree in scale.
EXP_A16 = 184.6649652337873
EXP_B16A = 16256.0 - 105.25
EXP_B16B = 16256.0 - 105.25 - 64.0


def _build_nc(n_tokens):
    import bass_rust
    import concourse.bass as bass
    import concourse.tile as tile
    import concourse.mybir as mybir

    f32 = mybir.dt.float32
    f32r = mybir.dt.float32r
    bf16 = mybir.dt.bfloat16
    i16 = mybir.dt.int16
    AF = mybir.ActivationFunctionType
    ADD = mybir.AluOpType.add
    MULT = mybir.AluOpType.mult
    N = n_tokens
    NQB = N // 512          # query blocks of 512
    NKB = N // 128          # key blocks of 128

    nc = bass.Bass("TRN2", target_bir_lowering=False, debug=False)

    x_d = nc.dram_tensor("x", [P, N], f32r, kind="ExternalInput").ap()
    wq_d = nc.dram_tensor("wq_t", [P, P], f32r, kind="ExternalInput").ap()
    wk_d = nc.dram_tensor("wk_t", [P, P], f32r, kind="ExternalInput").ap()
    wv_d = nc.dram_tensor("wv_t", [P, P], f32r, kind="ExternalInput").ap()
    wo_d = [nc.dram_tensor(f"wo_{h}", [HD, P], f32r,
                           kind="ExternalInput").ap() for h in range(HEADS)]
    bq_d = nc.dram_tensor("bq", [P, 1], f32, kind="ExternalInput").ap()
    bk_d = nc.dram_tensor("bk", [P, 1], f32, kind="ExternalInput").ap()
    bo_d = nc.dram_tensor("bo", [P, 1], f32, kind="ExternalInput").ap()
    vones_d = nc.dram_tensor("vones", [P, P], bf16, kind="ExternalInput").ap()
    wones_d = nc.dram_tensor("wones", [1, HD], f32r, kind="ExternalInput").ap()
    out_d = nc.dram_tensor("out", [P, N], f32, kind="ExternalOutput").ap()

    def r32(ap):
        return ap.bitcast(f32r)

    with tile.TileContext(nc) as tc:
        _frees = []

        def ptile(shape, name, dt=None):
            t, f = tc.tile(shape, dt or f32, name=name)
            _frees.append(f)
            return t

        # ---- persistent SBUF tensors ----
        wq = ptile([P, P], "wq", f32r)
        wk = ptile([P, P], "wk", f32r)
        wv = ptile([P, P], "wv", f32r)
        wo = [ptile([HD, P], f"wo{h}", f32r) for h in range(HEADS)]
        bq = ptile([P, 1], "bq_t")
        bk = ptile([P, 1], "bk_t")
        bo = ptile([P, 1], "bo_t")
        ones32 = ptile([1, HD], "ones32", f32r)
        qf = ptile([P, N], "qf", bf16)
        kf = ptile([P, N], "kf", bf16)
        vaug = ptile([P, NKB * 132], "vaug", bf16)

        nc.sync.dma_start(wq[:], wq_d)
        nc.sync.dma_start(wk[:], wk_d)
        nc.sync.dma_start(wv[:], wv_d)
        for h in range(HEADS):
            nc.sync.dma_start(wo[h][:], wo_d[h])
        nc.sync.dma_start(bq[:], bq_d)
        nc.sync.dma_start(bk[:], bk_d)
        nc.sync.dma_start(bo[:], bo_d)
        nc.sync.dma_start(ones32[:], wones_d)
        # ones column per (kblk, head): col 32 of each 33-wide group
        ones_ap = vaug[:].rearrange("p (j c) -> p j c", c=33)[:, :, 32]
        nc.sync.dma_start(ones_ap, vones_d[:, 0:NKB * HEADS])

        xt = []
        _xfrees = []
        for i in range(NQB):
            t, xf = tc.tile([P, 512], f32r, name=f"x{i}")
            _xfrees.append(xf)
            nc.sync.dma_start(t[:], x_d[:, i * 512:(i + 1) * 512])
            xt.append(t)

        # ---- phase 1: projections ----
        with tc.tile_pool(name="pproj", bufs=4, space="PSUM") as pproj, \
             tc.tile_pool(name="vproj", bufs=2, space="PSUM") as vproj:
            for i in range(NQB):
                ps = pproj.tile([P, 512], f32, tag="ps")
                nc.tensor.matmul(ps[:], wq[:], xt[i][:],
                                 start=True, stop=True)
                nc.vector.tensor_scalar(
                    qf[:, i * 512:(i + 1) * 512], ps[:], bq[:], None, ADD)
                ps2 = pproj.tile([P, 512], f32, tag="ps")
                nc.tensor.matmul(ps2[:], wk[:], xt[i][:],
                                 start=True, stop=True)
                nc.vector.tensor_scalar(
                    kf[:, i * 512:(i + 1) * 512], ps2[:], bk[:], None, ADD)
            for k in range(NKB):
                vp = vproj.tile([P, 128], f32, tag="vp")
                xsl = xt[k // 4][:, (k % 4) * 128:(k % 4 + 1) * 128]
                nc.tensor.matmul(vp[:], xsl, wv[:],
                                 start=True, stop=True)
                dst = vaug[:, k * 132:(k + 1) * 132].rearrange(
                    "p (h c) -> p h c", h=HEADS)[:, :, 0:HD]
                src = vp[:].rearrange("p (h c) -> p h c", h=HEADS)
                nc.scalar.copy(dst, src)
        for xf in reversed(_xfrees):
            xf()

        # ---- phase 2: attention (+ fused output projection) ----
        # Ring of 4 separate 1-bank psum tiles: per-buffer dependency
        # tracking gives a true depth-4 pipeline (a single [P,2048] tile
        # with manual bank slices serializes QK(i+1) behind exp(i) because
        # Tile tracks dependencies at whole-tile granularity). Same for ot:
        # four per-head tiles instead of one 4-bank tile.
        with tc.tile_pool(name="rgp", bufs=4, space="PSUM") as rgp, \
             tc.tile_pool(name="otp", bufs=1, space="PSUM") as otp:

            def slot():
                st = rgp.tile([P, 512], f32, tag="s", name="st")
                return st[:]

            eu = [0]       # global emit-unit counter (paces lazy tail work)
            tail = []      # [due_unit, fn] pending tail steps

            def drain(force=False):
                while tail and (force or tail[0][0] <= eu[0]):
                    tail.pop(0)[1]()

            def push(due, fn):
                tail.append([due, fn])
                tail.sort(key=lambda e: e[0])

            gx = [0]       # global unit counter for ACT/DVE exp split

            def _tail_head(h, q, ot_h, of_t):
                e = eu[0]
                zt = zp.tile([1, 512], f32, tag="z")
                rq = rp.tile([1, 512], f32, tag="r")
                rqr = rp.tile([1, 512], f32r, tag="rr")
                rsb = rsp.tile([HD, 512], f32, tag="rsb")
                oft = ofp.tile([HD, 512], f32r, tag="of")
                of_t[h] = oft

                def s_dma():
                    nc.vector.tensor_copy(zt[:], ot_h[32:33, :])

                def s_recip():
                    nc.vector.reciprocal(rq[:], zt[:])
                    nc.vector.tensor_copy(rqr[:], rq[:])

                def s_rbcast():
                    sl = slot()[0:HD, :]
                    nc.tensor.matmul(sl, ones32[:], rqr[:],
                                     start=True, stop=True,
                                     skip_group_check=True)

                    def s_rcopy():
                        nc.scalar.copy(rsb[:], sl)
                    push(eu[0] + 1, s_rcopy)

                    def s_ofmul():
                        nc.vector.tensor_mul(oft[:], ot_h[0:HD, :], rsb[:])
                    push(eu[0] + 2, s_ofmul)
                    if h == HEADS - 1:
                        push(eu[0] + 4, lambda: _oproj(q, of_t))

                push(e + 1, s_dma)
                push(e + 2, s_recip)
                push(e + 10, s_rbcast)

            def _oproj(q, of_t):
                po = slot()
                for h in range(HEADS):
                    nc.tensor.matmul(po, wo[h][:], of_t[h][:],
                                     start=(h == 0), stop=(h == HEADS - 1),
                                     skip_group_check=True)
                ob = obp.tile([P, 512], f32, tag="ob")
                nc.vector.tensor_scalar(ob[:], po, bo[:], None, ADD)
                nc.sync.dma_start(out_d[:, q * 512:(q + 1) * 512], ob[:])

            def emit_pv(pt, k, h, q, ot_l, of_t):
                c0 = k * 132 + 33 * h
                lhs = vaug[:, c0:c0 + 33]
                nc.tensor.matmul(
                    ot_l[h][:], lhs, pt[:],
                    start=(k == 0), stop=(k == NKB - 1),
                    skip_group_check=True)
                if k == NKB - 1:
                    _tail_head(h, q, ot_l[h], of_t)

            with tc.tile_pool(name="ptp", bufs=16) as ptp, \
                 tc.tile_pool(name="sap", bufs=8) as sap, \
                 tc.tile_pool(name="zp", bufs=6) as zp, \
                 tc.tile_pool(name="rp", bufs=6) as rp, \
                 tc.tile_pool(name="rsp", bufs=6) as rsp, \
                 tc.tile_pool(name="ofp", bufs=6) as ofp, \
                 tc.tile_pool(name="obp", bufs=2) as obp:

                for q in range(NQB):
                    qs = slice(q * 512, (q + 1) * 512)
                    ot_l = [otp.tile([33, 512], f32, tag=f"ot{h}",
                                     name=f"ot{h}")
                            for h in range(HEADS)]
                    of_t = [None] * HEADS
                    pvq = []
                    for k in range(NKB):
                        for hp in range(2):
                            hh = (2 * hp, 2 * hp + 1)
                            # two QK in adjacent PE slots, different 32-row
                            # bands -> they dual-issue on the PE
                            ss = []
                            for h in hh:
                                s = slot()
                                nc.tensor.matmul(
                                    s,
                                    kf[HD * h:HD * (h + 1),
                                       k * 128:(k + 1) * 128],
                                    qf[HD * h:HD * (h + 1), qs],
                                    start=True, stop=True,
                                    tile_position=(HD * h, 0))
                                ss.append(s)
                            for h, s in zip(hh, ss):
                                pt = ptp.tile([P, 512], bf16, tag="pt",
                                              name="pt")
                                if gx[0] % 13 < 9:
                                    nc.scalar.activation(pt[:], s, AF.Exp)
                                else:
                                    sa = sap.tile([P, 512], bf16, tag="sa")
                                    sb = sap.tile([P, 512], bf16, tag="sb")
                                    nc.vector.tensor_scalar(
                                        sa[:].bitcast(i16), s, EXP_A16,
                                        EXP_B16A, MULT, ADD)
                                    nc.vector.tensor_scalar(
                                        sb[:].bitcast(i16), s, EXP_A16,
                                        EXP_B16B, MULT, ADD)
                                    nc.gpsimd.tensor_tensor(
                                        pt[:], sa[:], sb[:], ADD)
                                gx[0] += 1
                                pvq.append((pt, k, h))
                            for _ in range(2):
                                if len(pvq) > 14:
                                    emit_pv(*pvq.pop(0), q, ot_l, of_t)
                                eu[0] += 1
                                drain()
                    while pvq:
                        emit_pv(*pvq.pop(0), q, ot_l, of_t)
                        eu[0] += 1
                        drain()
                drain(force=True)

        for f in reversed(_frees):
            f()

    _split_multiwait_drains(nc, mybir, bass_rust)
    return nc


def prep_weights(Wq, bq, Wk, bk, Wv, bv, Wo, bo):
    """Host-side weight preprocessing (all fp32 numpy)."""
    s = np.float32(1.0 / np.sqrt(HD))
    d = dict(
        wq_t=np.ascontiguousarray((s * Wq).T),
        bq=np.ascontiguousarray((s * bq).reshape(P, 1)),
        wk_t=np.ascontiguousarray(Wk.T),
        bk=np.ascontiguousarray(bk.reshape(P, 1)),
        wv_t=np.ascontiguousarray(Wv.T),
        bo=np.ascontiguousarray((bo + Wo @ bv).reshape(P, 1)).astype(
            np.float32),
    )
    import ml_dtypes
    d["vones"] = np.ones((P, P), ml_dtypes.bfloat16)
    d["wones"] = np.ones((1, HD), np.float32)
    for h in range(HEADS):
        d[f"wo_{h}"] = np.ascontiguousarray(Wo[:, HD * h:HD * (h + 1)].T)
    return d


LAST_RESULTS = None


def kernel(x, Wq, bq, Wk, bk, Wv, bv, Wo, bo):
    global LAST_RESULTS
    import os
    from concourse.bass_utils import run_bass_kernel_spmd

    x = np.asarray(x, np.float32)
    B, C, H, W = x.shape
    N = H * W
    key = ("nc", N)
    if key not in _CACHE:
        _CACHE[key] = _build_nc(N)
    nc = _CACHE[key]

    wmap = prep_weights(np.asarray(Wq, np.float32), np.asarray(bq, np.float32),
                        np.asarray(Wk, np.float32), np.asarray(bk, np.float32),
                        np.asarray(Wv, np.float32), np.asarray(bv, np.float32),
                        np.asarray(Wo, np.float32), np.asarray(bo, np.float32))

    in_maps = []
    for b in range(B):
        m = dict(wmap)
        m["x"] = np.ascontiguousarray(x[b].reshape(C, N))
        in_maps.append(m)

    tmpdir = os.environ.get("KERNEL_TMPDIR") or None
    res = run_bass_kernel_spmd(nc, in_maps, core_ids=list(range(B)),
                               tmpdir=tmpdir)
    LAST_RESULTS = res
    out = np.stack([res.results[b]["out"] for b in range(B)], axis=0)
    return out.reshape(B, C, H, W).astype(np.float32)


# revision 22
# speedup vs baseline: 1.0256x; 1.0256x over previous
"""MultiHeadSelfAttention2D Trainium2 kernel.

Full inputs -> shard batch (B=8) across 8 NeuronCores (1 image per core) ->
bass/Tile flash-attention-style kernel per core -> gather.

Per-core dataflow (feature-major, C=128 partitions, N=4096 tokens):
  Qf = (s*Wq) @ x + s*bq     (128 x N)   s = 1/sqrt(head_dim), folded on host
  Kf = Wk @ x + bk           (128 x N)
  Vaug = token-major V per (key-block, head): [V_h(32) | 1] so the PV matmul
         also accumulates the softmax denominator Z in psum row 32.
  Unit order is HEAD-major: for h in 4: for k in 32 key-blocks:
      S    = Kf_h[k].T @ Qf_h[qblk]      (PE, psum ring slot, 128x512)
      P    = exp(S)                      (split across ACT engine [exact Exp]
                                          and DVE [Schraudolph bit-trick exp:
                                          bf16бит = round(A*x+B) as int16])
      O_h += Vaug_hk . P                 (PE, accumulated over k into ot bank)
  After each head's last key-block (so the normalize pipeline of head h
  overlaps the attention of head h+1):
      Z_h row -> SBUF via DMA; 1/Z via 2-op approx reciprocal (DVE);
      partition-broadcast of 1/Z via tiny PE matmul (ones[1x32] outer);
      ACT copies broadcast psum->SBUF; DVE multiplies O_h * (1/Z) -> of_h.
  Per qblk tail: out += Wo_h.T @ of_h (4 accumulating f32r matmuls) + bias.

All tail work is emitted lazily with per-step due-units so the in-order PE
queue never blocks on the normalize chain latency.
"""

import numpy as np

EMBED = 128
HEADS = 4
HD = 32
P = 128

_CACHE = {}

# This container's walrus build only accepts one sync-wait per Drain
# instruction; Tile's tail drain carries one wait per live proc. Split the
# extra waits onto standalone EventSemaphore instructions (same engine,
# earlier in program order - semantically identical).
_DRAIN_MAX_WAITS = 1


def _split_multiwait_drains(nc, mybir, bass_rust):
    for fn in nc.m.functions:
        for bb in fn.blocks:
            new = []
            changed = False
            for inst in bb.instructions:
                si = inst.sync_info
                if (si is not None
                        and len(si.on_wait) > _DRAIN_MAX_WAITS):
                    changed = True
                    waits = list(si.on_wait)
                    for j, w in enumerate(waits[_DRAIN_MAX_WAITS:]):
                        es = mybir.InstEventSemaphore(
                            name=f"{inst.name}-wsplit{j}", ins=[], outs=[])
                        es.engine = inst.engine
                        es.sync_info = bass_rust.SyncInfo(
                            on_wait=[w], on_update=[])
                        nc.register_instruction(es)
                        new.append(es)
                    inst.sync_info = bass_rust.SyncInfo(
                        on_wait=waits[:_DRAIN_MAX_WAITS],
                        on_update=list(si.on_update))
                new.append(inst)
            if changed:
                bb.instructions = new


# Schraudolph exp in bf16 bit space: bf16bits(exp(x)) ~ round(A*x + B).
# Two evaluations half a sawtooth period apart (B and B-64) summed with a
# plain ADD cancel most of the linearization sawtooth (+-0.5% vs +-3%): the
# bit-space -64 shift is simultaneously the half-period phase shift AND the
# 2^-1/2 mixture weight. The -105.25 offset folds the constant factor so
# ACT-exp blocks and DVE-exp blocks agree in scale.
EXP_A16 = 184.6649652337873
EXP_B16A = 16256.0 - 105.25
EXP_B16B = 16256.0 - 105.25 - 64.0


def _build_nc(n_tokens):
    import bass_rust
    import concourse.bass as bass
    import concourse.tile as tile
    import concourse.mybir as mybir

    f32 = mybir.dt.float32
    f32r = mybir.dt.float32r
    bf16 = mybir.dt.bfloat16
    i16 = mybir.dt.int16
    AF = mybir.ActivationFunctionType
    ADD = mybir.AluOpType.add
    MULT = mybir.AluOpType.mult
    N = n_tokens
    NQB = N // 512          # query blocks of 512
    NKB = N // 128          # key blocks of 128

    nc = bass.Bass("TRN2", target_bir_lowering=False, debug=False)

    x_d = nc.dram_tensor("x", [P, N], f32r, kind="ExternalInput").ap()
    wq_d = nc.dram_tensor("wq_t", [P, P], f32r, kind="ExternalInput").ap()
    wk_d = nc.dram_tensor("wk_t", [P, P], f32r, kind="ExternalInput").ap()
    wv_d = nc.dram_tensor("wv_t", [P, P], f32r, kind="ExternalInput").ap()
    wo_d = [nc.dram_tensor(f"wo_{h}", [HD, P], f32r,
                           kind="ExternalInput").ap() for h in range(HEADS)]
    bq_d = nc.dram_tensor("bq", [P, 1], f32, kind="ExternalInput").ap()
    bk_d = nc.dram_tensor("bk", [P, 1], f32, kind="ExternalInput").ap()
    bo_d = nc.dram_tensor("bo", [P, 1], f32, kind="ExternalInput").ap()
    vones_d = nc.dram_tensor("vones", [P, P], bf16, kind="ExternalInput").ap()
    wones_d = nc.dram_tensor("wones", [1, HD], f32r, kind="ExternalInput").ap()
    out_d = nc.dram_tensor("out", [P, N], f32, kind="ExternalOutput").ap()

    def r32(ap):
        return ap.bitcast(f32r)

    with tile.TileContext(nc) as tc:
        _frees = []

        def ptile(shape, name, dt=None):
            t, f = tc.tile(shape, dt or f32, name=name)
            _frees.append(f)
            return t

        # ---- persistent SBUF tensors ----
        wq = ptile([P, P], "wq", f32r)
        wk = ptile([P, P], "wk", f32r)
        wv = ptile([P, P], "wv", f32r)
        wo = [ptile([HD, P], f"wo{h}", f32r) for h in range(HEADS)]
        bq = ptile([P, 1], "bq_t")
        bk = ptile([P, 1], "bk_t")
        bo = ptile([P, 1], "bo_t")
        ones32 = ptile([1, HD], "ones32", f32r)
        qf = ptile([P, N], "qf", bf16)
        kf = ptile([P, N], "kf", bf16)
        vaug = ptile([P, NKB * 132], "vaug", bf16)

        nc.sync.dma_start(wq[:], wq_d)
        nc.sync.dma_start(wk[:], wk_d)
        nc.sync.dma_start(wv[:], wv_d)
        for h in range(HEADS):
            nc.sync.dma_start(wo[h][:], wo_d[h])
        nc.sync.dma_start(bq[:], bq_d)
        nc.sync.dma_start(bk[:], bk_d)
        nc.sync.dma_start(bo[:], bo_d)
        nc.sync.dma_start(ones32[:], wones_d)
        # ones column per (kblk, head): col 32 of each 33-wide group
        ones_ap = vaug[:].rearrange("p (j c) -> p j c", c=33)[:, :, 32]
        nc.sync.dma_start(ones_ap, vones_d[:, 0:NKB * HEADS])

        xt = []
        _xfrees = []
        for i in range(NQB):
            t, xf = tc.tile([P, 512], f32r, name=f"x{i}")
            _xfrees.append(xf)
            nc.sync.dma_start(t[:], x_d[:, i * 512:(i + 1) * 512])
            xt.append(t)

        # ---- phase 1: projections ----
        with tc.tile_pool(name="pproj", bufs=4, space="PSUM") as pproj, \
             tc.tile_pool(name="vproj", bufs=2, space="PSUM") as vproj:
            for i in range(NQB):
                ps = pproj.tile([P, 512], f32, tag="ps")
                nc.tensor.matmul(ps[:], wq[:], xt[i][:],
                                 start=True, stop=True)
                nc.vector.tensor_scalar(
                    qf[:, i * 512:(i + 1) * 512], ps[:], bq[:], None, ADD)
                ps2 = pproj.tile([P, 512], f32, tag="ps")
                nc.tensor.matmul(ps2[:], wk[:], xt[i][:],
                                 start=True, stop=True)
                nc.vector.tensor_scalar(
                    kf[:, i * 512:(i + 1) * 512], ps2[:], bk[:], None, ADD)
            for k in range(NKB):
                vp = vproj.tile([P, 128], f32, tag="vp")
                xsl = xt[k // 4][:, (k % 4) * 128:(k % 4 + 1) * 128]
                nc.tensor.matmul(vp[:], xsl, wv[:],
                                 start=True, stop=True)
                dst = vaug[:, k * 132:(k + 1) * 132].rearrange(
                    "p (h c) -> p h c", h=HEADS)[:, :, 0:HD]
                src = vp[:].rearrange("p (h c) -> p h c", h=HEADS)
                nc.scalar.copy(dst, src)
        for xf in reversed(_xfrees):
            xf()

        # ---- phase 2: attention (+ fused output projection) ----
        # Ring of 4 separate 1-bank psum tiles: per-buffer dependency
        # tracking gives a true depth-4 pipeline (a single [P,2048] tile
        # with manual bank slices serializes QK(i+1) behind exp(i) because
        # Tile tracks dependencies at whole-tile granularity). Same for ot:
        # four per-head tiles instead of one 4-bank tile.
        with tc.tile_pool(name="rgp", bufs=2, space="PSUM") as rgp, \
             tc.tile_pool(name="otp", bufs=1, space="PSUM") as otp:

            def pairslot():
                st = rgp.tile([P, 1024], f32, tag="sp", name="sp")
                return st[:]

            def slot():
                return pairslot()[:, 0:512]

            eu = [0]       # global emit-unit counter (paces lazy tail work)
            tail = []      # [due_unit, fn] pending tail steps

            def drain(force=False):
                while tail and (force or tail[0][0] <= eu[0]):
                    tail.pop(0)[1]()

            def push(due, fn):
                tail.append([due, fn])
                tail.sort(key=lambda e: e[0])

            gx = [0]       # global unit counter for ACT/DVE exp split

            def _tail_head(h, q, ot_h, of_t):
                e = eu[0]
                zt = zp.tile([1, 512], f32, tag="z")
                rq = rp.tile([1, 512], f32, tag="r")
                rqr = rp.tile([1, 512], f32r, tag="rr")
                rsb = rsp.tile([HD, 512], f32, tag="rsb")
                oft = ofp.tile([HD, 512], f32r, tag="of")
                of_t[h] = oft

                def s_dma():
                    nc.vector.tensor_copy(zt[:], ot_h[32:33, :])

                def s_recip():
                    nc.vector.reciprocal(rq[:], zt[:])
                    nc.vector.tensor_copy(rqr[:], rq[:])

                def s_rbcast():
                    sl = slot()[0:HD, :]
                    nc.tensor.matmul(sl, ones32[:], rqr[:],
                                     start=True, stop=True,
                                     skip_group_check=True)

                    def s_rcopy():
                        nc.scalar.copy(rsb[:], sl)
                    push(eu[0] + 1, s_rcopy)

                    def s_ofmul():
                        nc.vector.tensor_mul(oft[:], ot_h[0:HD, :], rsb[:])
                    push(eu[0] + 2, s_ofmul)
                    if h == HEADS - 1:
                        push(eu[0] + 4, lambda: _oproj(q, of_t))

                push(e + 1, s_dma)
                push(e + 2, s_recip)
                push(e + 10, s_rbcast)

            def _oproj(q, of_t):
                po = slot()
                for h in range(HEADS):
                    nc.tensor.matmul(po, wo[h][:], of_t[h][:],
                                     start=(h == 0), stop=(h == HEADS - 1),
                                     skip_group_check=True)
                ob = obp.tile([P, 512], f32, tag="ob")
                nc.vector.tensor_scalar(ob[:], po, bo[:], None, ADD)
                nc.sync.dma_start(out_d[:, q * 512:(q + 1) * 512], ob[:])

            def emit_pv(pt_ap, k, h, q, ot_l, of_t):
                c0 = k * 132 + 33 * h
                lhs = vaug[:, c0:c0 + 33]
                nc.tensor.matmul(
                    ot_l[h][:], lhs, pt_ap,
                    start=(k == 0), stop=(k == NKB - 1),
                    skip_group_check=True)
                if k == NKB - 1:
                    _tail_head(h, q, ot_l[h], of_t)

            with tc.tile_pool(name="ptp", bufs=9) as ptp, \
                 tc.tile_pool(name="sap", bufs=8) as sap, \
                 tc.tile_pool(name="zp", bufs=6) as zp, \
                 tc.tile_pool(name="rp", bufs=6) as rp, \
                 tc.tile_pool(name="rsp", bufs=6) as rsp, \
                 tc.tile_pool(name="ofp", bufs=6) as ofp, \
                 tc.tile_pool(name="obp", bufs=2) as obp:

                for q in range(NQB):
                    qs = slice(q * 512, (q + 1) * 512)
                    ot_l = [otp.tile([33, 512], f32, tag=f"ot{h}",
                                     name=f"ot{h}")
                            for h in range(HEADS)]
                    of_t = [None] * HEADS
                    pvq = []
                    for k in range(NKB):
                        for hp in range(2):
                            hh = (2 * hp, 2 * hp + 1)
                            # two QK into one 2-bank pair tile; ONE exp op
                            # frees both slots at once so the scheduler
                            # keeps QK pairs adjacent (PE dual-issues
                            # row-disjoint 32-row bands)
                            sp2 = pairslot()
                            for j, h in enumerate(hh):
                                nc.tensor.matmul(
                                    sp2[:, j * 512:(j + 1) * 512],
                                    kf[HD * h:HD * (h + 1),
                                       k * 128:(k + 1) * 128],
                                    qf[HD * h:HD * (h + 1), qs],
                                    start=True, stop=True,
                                    tile_position=(HD * h, 0))
                            pt = ptp.tile([P, 1024], bf16, tag="pt",
                                          name="pt")
                            if gx[0] % 13 < 9:
                                nc.scalar.activation(pt[:], sp2, AF.Exp)
                            else:
                                sa = sap.tile([P, 1024], bf16, tag="sa")
                                sb = sap.tile([P, 1024], bf16, tag="sb")
                                nc.vector.tensor_scalar(
                                    sa[:].bitcast(i16), sp2, EXP_A16,
                                    EXP_B16A, MULT, ADD)
                                nc.vector.tensor_scalar(
                                    sb[:].bitcast(i16), sp2, EXP_A16,
                                    EXP_B16B, MULT, ADD)
                                nc.gpsimd.tensor_tensor(
                                    pt[:], sa[:], sb[:], ADD)
                            gx[0] += 1
                            for j, h in enumerate(hh):
                                pvq.append(
                                    (pt[:, j * 512:(j + 1) * 512], k, h))
                            for _ in range(2):
                                if len(pvq) > 14:
                                    emit_pv(*pvq.pop(0), q, ot_l, of_t)
                                eu[0] += 1
                                drain()
                    while pvq:
                        emit_pv(*pvq.pop(0), q, ot_l, of_t)
                        eu[0] += 1
                        drain()
                drain(force=True)

        for f in reversed(_frees):
            f()

    _split_multiwait_drains(nc, mybir, bass_rust)
    return nc


def prep_weights(Wq, bq, Wk, bk, Wv, bv, Wo, bo):
    """Host-side weight preprocessing (all fp32 numpy)."""
    s = np.float32(1.0 / np.sqrt(HD))
    d = dict(
        wq_t=np.ascontiguousarray((s * Wq).T),
        bq=np.ascontiguousarray((s * bq).reshape(P, 1)),
        wk_t=np.ascontiguousarray(Wk.T),
        bk=np.ascontiguousarray(bk.reshape(P, 1)),
        wv_t=np.ascontiguousarray(Wv.T),
        bo=np.ascontiguousarray((bo + Wo @ bv).reshape(P, 1)).astype(
            np.float32),
    )
    import ml_dtypes
    d["vones"] = np.ones((P, P), ml_dtypes.bfloat16)
    d["wones"] = np.ones((1, HD), np.float32)
    for h in range(HEADS):
        d[f"wo_{h}"] = np.ascontiguousarray(Wo[:, HD * h:HD * (h + 1)].T)
    return d


LAST_RESULTS = None


def kernel(x, Wq, bq, Wk, bk, Wv, bv, Wo, bo):
    global LAST_RESULTS
    import os
    from concourse.bass_utils import run_bass_kernel_spmd

    x = np.asarray(x, np.float32)
    B, C, H, W = x.shape
    N = H * W
    key = ("nc", N)
    if key not in _CACHE:
        _CACHE[key] = _build_nc(N)
    nc = _CACHE[key]

    wmap = prep_weights(np.asarray(Wq, np.float32), np.asarray(bq, np.float32),
                        np.asarray(Wk, np.float32), np.asarray(bk, np.float32),
                        np.asarray(Wv, np.float32), np.asarray(bv, np.float32),
                        np.asarray(Wo, np.float32), np.asarray(bo, np.float32))

    in_maps = []
    for b in range(B):
        m = dict(wmap)
        m["x"] = np.ascontiguousarray(x[b].reshape(C, N))
        in_maps.append(m)

    tmpdir = os.environ.get("KERNEL_TMPDIR") or None
    res = run_bass_kernel_spmd(nc, in_maps, core_ids=list(range(B)),
                               tmpdir=tmpdir)
    LAST_RESULTS = res
    out = np.stack([res.results[b]["out"] for b in range(B)], axis=0)
    return out.reshape(B, C, H, W).astype(np.float32)


# revision 23
# speedup vs baseline: 1.3384x; 1.3051x over previous
"""MultiHeadSelfAttention2D Trainium2 kernel.

Full inputs -> shard batch (B=8) across 8 NeuronCores (1 image per core) ->
bass/Tile flash-attention-style kernel per core -> gather.

Per-core dataflow (feature-major, C=128 partitions, N=4096 tokens):
  Qf = (s*Wq) @ x + s*bq     (128 x N)   s = 1/sqrt(head_dim), folded on host
  Kf = Wk @ x + bk           (128 x N)
  Vaug = token-major V per (key-block, head): [V_h(32) | 1] so the PV matmul
         also accumulates the softmax denominator Z in psum row 32.
  Unit order is HEAD-major: for h in 4: for k in 32 key-blocks:
      S    = Kf_h[k].T @ Qf_h[qblk]      (PE, psum ring slot, 128x512)
      P    = exp(S)                      (split across ACT engine [exact Exp]
                                          and DVE [Schraudolph bit-trick exp:
                                          bf16бит = round(A*x+B) as int16])
      O_h += Vaug_hk . P                 (PE, accumulated over k into ot bank)
  After each head's last key-block (so the normalize pipeline of head h
  overlaps the attention of head h+1):
      Z_h row -> SBUF via DMA; 1/Z via 2-op approx reciprocal (DVE);
      partition-broadcast of 1/Z via tiny PE matmul (ones[1x32] outer);
      ACT copies broadcast psum->SBUF; DVE multiplies O_h * (1/Z) -> of_h.
  Per qblk tail: out += Wo_h.T @ of_h (4 accumulating f32r matmuls) + bias.

All tail work is emitted lazily with per-step due-units so the in-order PE
queue never blocks on the normalize chain latency.
"""

import numpy as np

EMBED = 128
HEADS = 4
HD = 32
P = 128

_CACHE = {}

# This container's walrus build only accepts one sync-wait per Drain
# instruction; Tile's tail drain carries one wait per live proc. Split the
# extra waits onto standalone EventSemaphore instructions (same engine,
# earlier in program order - semantically identical).
_DRAIN_MAX_WAITS = 1


def _split_multiwait_drains(nc, mybir, bass_rust):
    for fn in nc.m.functions:
        for bb in fn.blocks:
            new = []
            changed = False
            for inst in bb.instructions:
                si = inst.sync_info
                if (si is not None
                        and len(si.on_wait) > _DRAIN_MAX_WAITS):
                    changed = True
                    waits = list(si.on_wait)
                    for j, w in enumerate(waits[_DRAIN_MAX_WAITS:]):
                        es = mybir.InstEventSemaphore(
                            name=f"{inst.name}-wsplit{j}", ins=[], outs=[])
                        es.engine = inst.engine
                        es.sync_info = bass_rust.SyncInfo(
                            on_wait=[w], on_update=[])
                        nc.register_instruction(es)
                        new.append(es)
                    inst.sync_info = bass_rust.SyncInfo(
                        on_wait=waits[:_DRAIN_MAX_WAITS],
                        on_update=list(si.on_update))
                new.append(inst)
            if changed:
                bb.instructions = new


# Schraudolph exp in bf16 bit space: bf16bits(exp(x)) ~ round(A*x + B).
# Two evaluations half a sawtooth period apart (B and B-64) summed with a
# plain ADD cancel most of the linearization sawtooth (+-0.5% vs +-3%): the
# bit-space -64 shift is simultaneously the half-period phase shift AND the
# 2^-1/2 mixture weight. The -105.25 offset folds the constant factor so
# ACT-exp blocks and DVE-exp blocks agree in scale.
EXP_A16 = 184.6649652337873
EXP_B16A = 16256.0 - 105.25
EXP_B16B = 16256.0 - 105.25 - 64.0


def _build_nc(n_tokens):
    import bass_rust
    import concourse.bass as bass
    import concourse.tile as tile
    import concourse.mybir as mybir

    f32 = mybir.dt.float32
    f32r = mybir.dt.float32r
    bf16 = mybir.dt.bfloat16
    i16 = mybir.dt.int16
    AF = mybir.ActivationFunctionType
    ADD = mybir.AluOpType.add
    MULT = mybir.AluOpType.mult
    N = n_tokens
    NQB = N // 512          # query blocks of 512
    NKB = N // 128          # key blocks of 128

    nc = bass.Bass("TRN2", target_bir_lowering=False, debug=False)

    x_d = nc.dram_tensor("x", [P, N], f32r, kind="ExternalInput").ap()
    wq_d = nc.dram_tensor("wq_t", [P, P], f32r, kind="ExternalInput").ap()
    wk_d = nc.dram_tensor("wk_t", [P, P], f32r, kind="ExternalInput").ap()
    wv_d = nc.dram_tensor("wv_t", [P, P], f32r, kind="ExternalInput").ap()
    wo_d = [nc.dram_tensor(f"wo_{h}", [HD, P], f32r,
                           kind="ExternalInput").ap() for h in range(HEADS)]
    bq_d = nc.dram_tensor("bq", [P, 1], f32, kind="ExternalInput").ap()
    bk_d = nc.dram_tensor("bk", [P, 1], f32, kind="ExternalInput").ap()
    bo_d = nc.dram_tensor("bo", [P, 1], f32, kind="ExternalInput").ap()
    vones_d = nc.dram_tensor("vones", [P, P], bf16, kind="ExternalInput").ap()
    wones_d = nc.dram_tensor("wones", [1, HD], f32r, kind="ExternalInput").ap()
    out_d = nc.dram_tensor("out", [P, N], f32, kind="ExternalOutput").ap()

    def r32(ap):
        return ap.bitcast(f32r)

    with tile.TileContext(nc) as tc:
        _frees = []

        def ptile(shape, name, dt=None):
            t, f = tc.tile(shape, dt or f32, name=name)
            _frees.append(f)
            return t

        # ---- persistent SBUF tensors ----
        wq = ptile([P, P], "wq", f32r)
        wk = ptile([P, P], "wk", f32r)
        wv = ptile([P, P], "wv", f32r)
        wo = [ptile([HD, P], f"wo{h}", f32r) for h in range(HEADS)]
        bq = ptile([P, 1], "bq_t")
        bk = ptile([P, 1], "bk_t")
        bo = ptile([P, 1], "bo_t")
        ones32 = ptile([1, HD], "ones32", f32r)
        qf = ptile([P, N], "qf", bf16)
        kf = ptile([P, N], "kf", bf16)
        vaug = ptile([P, NKB * 132], "vaug", bf16)

        nc.sync.dma_start(wq[:], wq_d)
        nc.sync.dma_start(wk[:], wk_d)
        nc.sync.dma_start(wv[:], wv_d)
        for h in range(HEADS):
            nc.sync.dma_start(wo[h][:], wo_d[h])
        nc.sync.dma_start(bq[:], bq_d)
        nc.sync.dma_start(bk[:], bk_d)
        nc.sync.dma_start(bo[:], bo_d)
        nc.sync.dma_start(ones32[:], wones_d)
        # ones column per (kblk, head): col 32 of each 33-wide group
        ones_ap = vaug[:].rearrange("p (j c) -> p j c", c=33)[:, :, 32]
        nc.sync.dma_start(ones_ap, vones_d[:, 0:NKB * HEADS])

        xt = []
        _xfrees = []
        for i in range(NQB):
            t, xf = tc.tile([P, 512], f32r, name=f"x{i}")
            _xfrees.append(xf)
            nc.sync.dma_start(t[:], x_d[:, i * 512:(i + 1) * 512])
            xt.append(t)

        # ---- phase 1: projections ----
        with tc.tile_pool(name="pproj", bufs=4, space="PSUM") as pproj, \
             tc.tile_pool(name="vproj", bufs=2, space="PSUM") as vproj:
            for i in range(NQB):
                ps = pproj.tile([P, 512], f32, tag="ps")
                nc.tensor.matmul(ps[:], wq[:], xt[i][:],
                                 start=True, stop=True)
                nc.vector.tensor_scalar(
                    qf[:, i * 512:(i + 1) * 512], ps[:], bq[:], None, ADD)
                ps2 = pproj.tile([P, 512], f32, tag="ps")
                nc.tensor.matmul(ps2[:], wk[:], xt[i][:],
                                 start=True, stop=True)
                nc.vector.tensor_scalar(
                    kf[:, i * 512:(i + 1) * 512], ps2[:], bk[:], None, ADD)
            for k in range(NKB):
                vp = vproj.tile([P, 128], f32, tag="vp")
                xsl = xt[k // 4][:, (k % 4) * 128:(k % 4 + 1) * 128]
                nc.tensor.matmul(vp[:], xsl, wv[:],
                                 start=True, stop=True)
                dst = vaug[:, k * 132:(k + 1) * 132].rearrange(
                    "p (h c) -> p h c", h=HEADS)[:, :, 0:HD]
                src = vp[:].rearrange("p (h c) -> p h c", h=HEADS)
                nc.scalar.copy(dst, src)
        for xf in reversed(_xfrees):
            xf()

        # ---- phase 2: attention (+ fused output projection) ----
        # Ring of 4 separate 1-bank psum tiles: per-buffer dependency
        # tracking gives a true depth-4 pipeline (a single [P,2048] tile
        # with manual bank slices serializes QK(i+1) behind exp(i) because
        # Tile tracks dependencies at whole-tile granularity). Same for ot:
        # four per-head tiles instead of one 4-bank tile.
        with tc.tile_pool(name="rgp", bufs=2, space="PSUM") as rgp, \
             tc.tile_pool(name="otp", bufs=1, space="PSUM") as otp:

            def pairslot():
                st = rgp.tile([P, 1024], f32, tag="sp", name="sp")
                return st[:]

            def slot():
                return pairslot()[:, 0:512]

            eu = [0]       # global emit-unit counter (paces lazy tail work)
            tail = []      # [due_unit, fn] pending tail steps

            def drain(force=False):
                while tail and (force or tail[0][0] <= eu[0]):
                    tail.pop(0)[1]()

            def push(due, fn):
                tail.append([due, fn])
                tail.sort(key=lambda e: e[0])

            gx = [0]       # global unit counter for ACT/DVE exp split

            def _tail_head(h, q, ot_h, of_t):
                e = eu[0]
                zt = zp.tile([1, 512], f32, tag="z")
                rq = rp.tile([1, 512], f32, tag="r")
                rqr = rp.tile([1, 512], f32r, tag="rr")
                rsb = rsp.tile([HD, 512], f32, tag="rsb")
                oft = ofp.tile([HD, 512], f32r, tag="of")
                of_t[h] = oft

                def s_dma():
                    nc.vector.tensor_copy(zt[:], ot_h[32:33, :])

                def s_recip():
                    nc.vector.reciprocal(rq[:], zt[:])
                    nc.vector.tensor_copy(rqr[:], rq[:])

                def s_rbcast():
                    sl = slot()[0:HD, :]
                    nc.tensor.matmul(sl, ones32[:], rqr[:],
                                     start=True, stop=True,
                                     skip_group_check=True)

                    def s_rcopy():
                        nc.scalar.copy(rsb[:], sl)
                    push(eu[0] + 1, s_rcopy)

                    def s_ofmul():
                        nc.vector.tensor_mul(oft[:], ot_h[0:HD, :], rsb[:])
                    push(eu[0] + 2, s_ofmul)
                    if h == HEADS - 1:
                        push(eu[0] + 4, lambda: _oproj(q, of_t))

                push(e + 1, s_dma)
                push(e + 2, s_recip)
                push(e + 10, s_rbcast)

            def _oproj(q, of_t):
                po = slot()
                for h in range(HEADS):
                    nc.tensor.matmul(po, wo[h][:], of_t[h][:],
                                     start=(h == 0), stop=(h == HEADS - 1),
                                     skip_group_check=True)
                ob = obp.tile([P, 512], f32, tag="ob")
                nc.vector.tensor_scalar(ob[:], po, bo[:], None, ADD)
                nc.sync.dma_start(out_d[:, q * 512:(q + 1) * 512], ob[:])

            def emit_pv(pt_ap, k, h, q, ot_l, of_t):
                c0 = k * 132 + 33 * h
                lhs = vaug[:, c0:c0 + 33]
                nc.tensor.matmul(
                    ot_l[h][:], lhs, pt_ap,
                    start=(k == 0), stop=(k == NKB - 1),
                    skip_group_check=True)
                if k == NKB - 1:
                    _tail_head(h, q, ot_l[h], of_t)

            with tc.tile_pool(name="ptp", bufs=9) as ptp, \
                 tc.tile_pool(name="sap", bufs=8) as sap, \
                 tc.tile_pool(name="zp", bufs=6) as zp, \
                 tc.tile_pool(name="rp", bufs=6) as rp, \
                 tc.tile_pool(name="rsp", bufs=6) as rsp, \
                 tc.tile_pool(name="ofp", bufs=6) as ofp, \
                 tc.tile_pool(name="obp", bufs=2) as obp:

                for q in range(NQB):
                    qs = slice(q * 512, (q + 1) * 512)
                    ot_l = [otp.tile([33, 512], f32, tag=f"ot{h}",
                                     name=f"ot{h}")
                            for h in range(HEADS)]
                    of_t = [None] * HEADS
                    pvq = []
                    for k in range(NKB):
                        for hp in range(2):
                            hh = (2 * hp, 2 * hp + 1)
                            # two QK into one 2-bank pair tile; ONE exp op
                            # frees both slots at once so the scheduler
                            # keeps QK pairs adjacent (PE dual-issues
                            # row-disjoint 32-row bands)
                            sp2 = pairslot()
                            for j, h in enumerate(hh):
                                nc.tensor.matmul(
                                    sp2[:, j * 512:(j + 1) * 512],
                                    kf[HD * h:HD * (h + 1),
                                       k * 128:(k + 1) * 128],
                                    qf[HD * h:HD * (h + 1), qs],
                                    start=True, stop=True,
                                    tile_position=(HD * h, 0))
                            pt = ptp.tile([P, 1024], bf16, tag="pt",
                                          name="pt")
                            if True:  # 1024-wide ACT exp fits under PE pace
                                nc.scalar.activation(pt[:], sp2, AF.Exp)
                            else:
                                sa = sap.tile([P, 1024], bf16, tag="sa")
                                sb = sap.tile([P, 1024], bf16, tag="sb")
                                nc.vector.tensor_scalar(
                                    sa[:].bitcast(i16), sp2, EXP_A16,
                                    EXP_B16A, MULT, ADD)
                                nc.vector.tensor_scalar(
                                    sb[:].bitcast(i16), sp2, EXP_A16,
                                    EXP_B16B, MULT, ADD)
                                nc.gpsimd.tensor_tensor(
                                    pt[:], sa[:], sb[:], ADD)
                            gx[0] += 1
                            for j, h in enumerate(hh):
                                pvq.append(
                                    (pt[:, j * 512:(j + 1) * 512], k, h))
                            for _ in range(2):
                                if len(pvq) > 14:
                                    emit_pv(*pvq.pop(0), q, ot_l, of_t)
                                eu[0] += 1
                                drain()
                    while pvq:
                        emit_pv(*pvq.pop(0), q, ot_l, of_t)
                        eu[0] += 1
                        drain()
                drain(force=True)

        for f in reversed(_frees):
            f()

    _split_multiwait_drains(nc, mybir, bass_rust)
    return nc


def prep_weights(Wq, bq, Wk, bk, Wv, bv, Wo, bo):
    """Host-side weight preprocessing (all fp32 numpy)."""
    s = np.float32(1.0 / np.sqrt(HD))
    d = dict(
        wq_t=np.ascontiguousarray((s * Wq).T),
        bq=np.ascontiguousarray((s * bq).reshape(P, 1)),
        wk_t=np.ascontiguousarray(Wk.T),
        bk=np.ascontiguousarray(bk.reshape(P, 1)),
        wv_t=np.ascontiguousarray(Wv.T),
        bo=np.ascontiguousarray((bo + Wo @ bv).reshape(P, 1)).astype(
            np.float32),
    )
    import ml_dtypes
    d["vones"] = np.ones((P, P), ml_dtypes.bfloat16)
    d["wones"] = np.ones((1, HD), np.float32)
    for h in range(HEADS):
        d[f"wo_{h}"] = np.ascontiguousarray(Wo[:, HD * h:HD * (h + 1)].T)
    return d


LAST_RESULTS = None


def kernel(x, Wq, bq, Wk, bk, Wv, bv, Wo, bo):
    global LAST_RESULTS
    import os
    from concourse.bass_utils import run_bass_kernel_spmd

    x = np.asarray(x, np.float32)
    B, C, H, W = x.shape
    N = H * W
    key = ("nc", N)
    if key not in _CACHE:
        _CACHE[key] = _build_nc(N)
    nc = _CACHE[key]

    wmap = prep_weights(np.asarray(Wq, np.float32), np.asarray(bq, np.float32),
                        np.asarray(Wk, np.float32), np.asarray(bk, np.float32),
                        np.asarray(Wv, np.float32), np.asarray(bv, np.float32),
                        np.asarray(Wo, np.float32), np.asarray(bo, np.float32))

    in_maps = []
    for b in range(B):
        m = dict(wmap)
        m["x"] = np.ascontiguousarray(x[b].reshape(C, N))
        in_maps.append(m)

    tmpdir = os.environ.get("KERNEL_TMPDIR") or None
    res = run_bass_kernel_spmd(nc, in_maps, core_ids=list(range(B)),
                               tmpdir=tmpdir)
    LAST_RESULTS = res
    out = np.stack([res.results[b]["out"] for b in range(B)], axis=0)
    return out.reshape(B, C, H, W).astype(np.float32)


# revision 24
# speedup vs baseline: 1.5439x; 1.1535x over previous
"""MultiHeadSelfAttention2D Trainium2 kernel.

Full inputs -> shard batch (B=8) across 8 NeuronCores (1 image per core) ->
bass/Tile flash-attention-style kernel per core -> gather.

Per-core dataflow (feature-major, C=128 partitions, N=4096 tokens):
  Qf = (s*Wq) @ x + s*bq     (128 x N)   s = 1/sqrt(head_dim), folded on host
  Kf = Wk @ x + bk           (128 x N)
  Vaug = token-major V per (key-block, head): [V_h(32) | 1] so the PV matmul
         also accumulates the softmax denominator Z in psum row 32.
  Unit order is HEAD-major: for h in 4: for k in 32 key-blocks:
      S    = Kf_h[k].T @ Qf_h[qblk]      (PE, psum ring slot, 128x512)
      P    = exp(S)                      (split across ACT engine [exact Exp]
                                          and DVE [Schraudolph bit-trick exp:
                                          bf16бит = round(A*x+B) as int16])
      O_h += Vaug_hk . P                 (PE, accumulated over k into ot bank)
  After each head's last key-block (so the normalize pipeline of head h
  overlaps the attention of head h+1):
      Z_h row -> SBUF via DMA; 1/Z via 2-op approx reciprocal (DVE);
      partition-broadcast of 1/Z via tiny PE matmul (ones[1x32] outer);
      ACT copies broadcast psum->SBUF; DVE multiplies O_h * (1/Z) -> of_h.
  Per qblk tail: out += Wo_h.T @ of_h (4 accumulating f32r matmuls) + bias.

All tail work is emitted lazily with per-step due-units so the in-order PE
queue never blocks on the normalize chain latency.
"""

import numpy as np

EMBED = 128
HEADS = 4
HD = 32
P = 128

_CACHE = {}

# This container's walrus build only accepts one sync-wait per Drain
# instruction; Tile's tail drain carries one wait per live proc. Split the
# extra waits onto standalone EventSemaphore instructions (same engine,
# earlier in program order - semantically identical).
_DRAIN_MAX_WAITS = 1


def _split_multiwait_drains(nc, mybir, bass_rust):
    for fn in nc.m.functions:
        for bb in fn.blocks:
            new = []
            changed = False
            for inst in bb.instructions:
                si = inst.sync_info
                if (si is not None
                        and len(si.on_wait) > _DRAIN_MAX_WAITS):
                    changed = True
                    waits = list(si.on_wait)
                    for j, w in enumerate(waits[_DRAIN_MAX_WAITS:]):
                        es = mybir.InstEventSemaphore(
                            name=f"{inst.name}-wsplit{j}", ins=[], outs=[])
                        es.engine = inst.engine
                        es.sync_info = bass_rust.SyncInfo(
                            on_wait=[w], on_update=[])
                        nc.register_instruction(es)
                        new.append(es)
                    inst.sync_info = bass_rust.SyncInfo(
                        on_wait=waits[:_DRAIN_MAX_WAITS],
                        on_update=list(si.on_update))
                new.append(inst)
            if changed:
                bb.instructions = new


# Schraudolph exp in bf16 bit space: bf16bits(exp(x)) ~ round(A*x + B).
# Two evaluations half a sawtooth period apart (B and B-64) summed with a
# plain ADD cancel most of the linearization sawtooth (+-0.5% vs +-3%): the
# bit-space -64 shift is simultaneously the half-period phase shift AND the
# 2^-1/2 mixture weight. The -105.25 offset folds the constant factor so
# ACT-exp blocks and DVE-exp blocks agree in scale.
EXP_A16 = 184.6649652337873
EXP_B16A = 16256.0 - 105.25
EXP_B16B = 16256.0 - 105.25 - 64.0


def _build_nc(n_tokens):
    import bass_rust
    import concourse.bass as bass
    import concourse.tile as tile
    import concourse.mybir as mybir

    f32 = mybir.dt.float32
    f32r = mybir.dt.float32r
    bf16 = mybir.dt.bfloat16
    i16 = mybir.dt.int16
    AF = mybir.ActivationFunctionType
    ADD = mybir.AluOpType.add
    MULT = mybir.AluOpType.mult
    N = n_tokens
    NQB = N // 512          # query blocks of 512
    NKB = N // 128          # key blocks of 128

    nc = bass.Bass("TRN2", target_bir_lowering=False, debug=False)

    x_d = nc.dram_tensor("x", [P, N], f32r, kind="ExternalInput").ap()
    wq_d = nc.dram_tensor("wq_t", [P, P], f32r, kind="ExternalInput").ap()
    wk_d = nc.dram_tensor("wk_t", [P, P], f32r, kind="ExternalInput").ap()
    wv_d = nc.dram_tensor("wv_t", [P, P], f32r, kind="ExternalInput").ap()
    wo_d = [nc.dram_tensor(f"wo_{h}", [HD, P], f32r,
                           kind="ExternalInput").ap() for h in range(HEADS)]
    bq_d = nc.dram_tensor("bq", [P, 1], f32, kind="ExternalInput").ap()
    bk_d = nc.dram_tensor("bk", [P, 1], f32, kind="ExternalInput").ap()
    bo_d = nc.dram_tensor("bo", [P, 1], f32, kind="ExternalInput").ap()
    vones_d = nc.dram_tensor("vones", [P, P], bf16, kind="ExternalInput").ap()
    wones_d = nc.dram_tensor("wones", [1, HD], f32r, kind="ExternalInput").ap()
    out_d = nc.dram_tensor("out", [P, N], f32, kind="ExternalOutput").ap()

    def r32(ap):
        return ap.bitcast(f32r)

    with tile.TileContext(nc) as tc:
        _frees = []

        def ptile(shape, name, dt=None):
            t, f = tc.tile(shape, dt or f32, name=name)
            _frees.append(f)
            return t

        # ---- persistent SBUF tensors ----
        wq = ptile([P, P], "wq", f32r)
        wk = ptile([P, P], "wk", f32r)
        wv = ptile([P, P], "wv", f32r)
        wo = [ptile([HD, P], f"wo{h}", f32r) for h in range(HEADS)]
        bq = ptile([P, 1], "bq_t")
        bk = ptile([P, 1], "bk_t")
        bo = ptile([P, 1], "bo_t")
        ones32 = ptile([1, HD], "ones32", f32r)
        qf = ptile([P, N], "qf", bf16)
        kf = ptile([P, N], "kf", bf16)
        vaug = ptile([P, NKB * 132], "vaug", bf16)

        nc.sync.dma_start(wq[:], wq_d)
        nc.sync.dma_start(wk[:], wk_d)
        nc.sync.dma_start(wv[:], wv_d)
        for h in range(HEADS):
            nc.sync.dma_start(wo[h][:], wo_d[h])
        nc.sync.dma_start(bq[:], bq_d)
        nc.sync.dma_start(bk[:], bk_d)
        nc.sync.dma_start(bo[:], bo_d)
        nc.sync.dma_start(ones32[:], wones_d)
        # ones column per (kblk, head): col 32 of each 33-wide group
        ones_ap = vaug[:].rearrange("p (j c) -> p j c", c=33)[:, :, 32]
        nc.sync.dma_start(ones_ap, vones_d[:, 0:NKB * HEADS])

        xt = []
        _xfrees = []
        for i in range(NQB):
            t, xf = tc.tile([P, 512], f32r, name=f"x{i}")
            _xfrees.append(xf)
            nc.sync.dma_start(t[:], x_d[:, i * 512:(i + 1) * 512])
            xt.append(t)

        # ---- phase 1: projections ----
        with tc.tile_pool(name="pproj", bufs=4, space="PSUM") as pproj, \
             tc.tile_pool(name="vproj", bufs=2, space="PSUM") as vproj:
            for i in range(NQB):
                ps = pproj.tile([P, 512], f32, tag="ps")
                nc.tensor.matmul(ps[:], wq[:], xt[i][:],
                                 start=True, stop=True)
                nc.vector.tensor_scalar(
                    qf[:, i * 512:(i + 1) * 512], ps[:], bq[:], None, ADD)
                ps2 = pproj.tile([P, 512], f32, tag="ps")
                nc.tensor.matmul(ps2[:], wk[:], xt[i][:],
                                 start=True, stop=True)
                nc.vector.tensor_scalar(
                    kf[:, i * 512:(i + 1) * 512], ps2[:], bk[:], None, ADD)
            for k in range(NKB):
                vp = vproj.tile([P, 128], f32, tag="vp")
                xsl = xt[k // 4][:, (k % 4) * 128:(k % 4 + 1) * 128]
                nc.tensor.matmul(vp[:], xsl, wv[:],
                                 start=True, stop=True)
                dst = vaug[:, k * 132:(k + 1) * 132].rearrange(
                    "p (h c) -> p h c", h=HEADS)[:, :, 0:HD]
                src = vp[:].rearrange("p (h c) -> p h c", h=HEADS)
                nc.scalar.copy(dst, src)
        for xf in reversed(_xfrees):
            xf()

        # ---- phase 2: attention (+ fused output projection) ----
        # Ring of 4 separate 1-bank psum tiles: per-buffer dependency
        # tracking gives a true depth-4 pipeline (a single [P,2048] tile
        # with manual bank slices serializes QK(i+1) behind exp(i) because
        # Tile tracks dependencies at whole-tile granularity). Same for ot:
        # four per-head tiles instead of one 4-bank tile.
        with tc.tile_pool(name="rgp", bufs=2, space="PSUM") as rgp, \
             tc.tile_pool(name="otp", bufs=1, space="PSUM") as otp:

            def pairslot():
                st = rgp.tile([P, 1024], f32, tag="sp", name="sp")
                return st[:]

            def slot():
                return pairslot()[:, 0:512]

            eu = [0]       # global emit-unit counter (paces lazy tail work)
            tail = []      # [due_unit, fn] pending tail steps

            def drain(force=False):
                while tail and (force or tail[0][0] <= eu[0]):
                    tail.pop(0)[1]()

            def push(due, fn):
                tail.append([due, fn])
                tail.sort(key=lambda e: e[0])

            gx = [0]       # global unit counter for ACT/DVE exp split

            def _tail_head(h, q, ot_h, of_t):
                e = eu[0]
                rq = rp.tile([1, 512], f32, tag="r")
                rqr = rp.tile([1, 512], f32r, tag="rr")
                rsb = rsp.tile([HD, 512], f32, tag="rsb")
                oft = ofp.tile([HD, 512], f32r, tag="of")
                of_t[h] = oft

                def s_recip():
                    nc.vector.reciprocal(rq[:], ot_h[32:33, :])
                    nc.vector.tensor_copy(rqr[:], rq[:])

                def s_rbcast():
                    sl = slot()[0:HD, :]
                    nc.tensor.matmul(sl, ones32[:], rqr[:],
                                     start=True, stop=True,
                                     skip_group_check=True)

                    def s_rcopy():
                        nc.vector.tensor_copy(rsb[:], sl)
                    push(eu[0] + 1, s_rcopy)

                    def s_ofmul():
                        nc.vector.tensor_mul(oft[:], ot_h[0:HD, :], rsb[:])
                    push(eu[0] + 2, s_ofmul)
                    if h == HEADS - 1:
                        push(eu[0] + 4, lambda: _oproj(q, of_t))

                push(e + 1, s_recip)
                push(e + 16, s_rbcast)

            def _oproj(q, of_t):
                po = slot()
                for h in range(HEADS):
                    nc.tensor.matmul(po, wo[h][:], of_t[h][:],
                                     start=(h == 0), stop=(h == HEADS - 1),
                                     skip_group_check=True)
                ob = obp.tile([P, 512], f32, tag="ob")
                nc.vector.tensor_scalar(ob[:], po, bo[:], None, ADD)
                nc.sync.dma_start(out_d[:, q * 512:(q + 1) * 512], ob[:])

            def emit_pv(pt_ap, k, h, q, ot_l, of_t):
                c0 = k * 132 + 33 * h
                lhs = vaug[:, c0:c0 + 33]
                nc.tensor.matmul(
                    ot_l[h][:], lhs, pt_ap,
                    start=(k == 0), stop=(k == NKB - 1),
                    skip_group_check=True)
                if k == NKB - 1:
                    _tail_head(h, q, ot_l[h], of_t)

            with tc.tile_pool(name="ptp", bufs=12) as ptp, \
                 tc.tile_pool(name="sap", bufs=8) as sap, \
                 tc.tile_pool(name="zp", bufs=6) as zp, \
                 tc.tile_pool(name="rp", bufs=6) as rp, \
                 tc.tile_pool(name="rsp", bufs=6) as rsp, \
                 tc.tile_pool(name="ofp", bufs=6) as ofp, \
                 tc.tile_pool(name="obp", bufs=2) as obp:

                for q in range(NQB):
                    qs = slice(q * 512, (q + 1) * 512)
                    ot_l = [otp.tile([33, 512], f32, tag=f"ot{h}",
                                     name=f"ot{h}")
                            for h in range(HEADS)]
                    of_t = [None] * HEADS
                    pvq = []
                    for k in range(NKB):
                        for hp in range(2):
                            hh = (2 * hp, 2 * hp + 1)
                            # two QK into one 2-bank pair tile; ONE exp op
                            # frees both slots at once so the scheduler
                            # keeps QK pairs adjacent (PE dual-issues
                            # row-disjoint 32-row bands)
                            sp2 = pairslot()
                            for j, h in enumerate(hh):
                                nc.tensor.matmul(
                                    sp2[:, j * 512:(j + 1) * 512],
                                    kf[HD * h:HD * (h + 1),
                                       k * 128:(k + 1) * 128],
                                    qf[HD * h:HD * (h + 1), qs],
                                    start=True, stop=True,
                                    tile_position=(HD * h, 0))
                            pt = ptp.tile([P, 1024], bf16, tag="pt",
                                          name="pt")
                            if True:  # 1024-wide ACT exp fits under PE pace
                                nc.scalar.activation(pt[:], sp2, AF.Exp)
                            else:
                                sa = sap.tile([P, 1024], bf16, tag="sa")
                                sb = sap.tile([P, 1024], bf16, tag="sb")
                                nc.vector.tensor_scalar(
                                    sa[:].bitcast(i16), sp2, EXP_A16,
                                    EXP_B16A, MULT, ADD)
                                nc.vector.tensor_scalar(
                                    sb[:].bitcast(i16), sp2, EXP_A16,
                                    EXP_B16B, MULT, ADD)
                                nc.gpsimd.tensor_tensor(
                                    pt[:], sa[:], sb[:], ADD)
                            gx[0] += 1
                            for j, h in enumerate(hh):
                                pvq.append(
                                    (pt[:, j * 512:(j + 1) * 512], k, h))
                            for _ in range(2):
                                if len(pvq) > 18:
                                    emit_pv(*pvq.pop(0), q, ot_l, of_t)
                                eu[0] += 1
                                drain()
                    while pvq:
                        emit_pv(*pvq.pop(0), q, ot_l, of_t)
                        eu[0] += 1
                        drain()
                drain(force=True)

        for f in reversed(_frees):
            f()

    _split_multiwait_drains(nc, mybir, bass_rust)
    return nc


def prep_weights(Wq, bq, Wk, bk, Wv, bv, Wo, bo):
    """Host-side weight preprocessing (all fp32 numpy)."""
    s = np.float32(1.0 / np.sqrt(HD))
    d = dict(
        wq_t=np.ascontiguousarray((s * Wq).T),
        bq=np.ascontiguousarray((s * bq).reshape(P, 1)),
        wk_t=np.ascontiguousarray(Wk.T),
        bk=np.ascontiguousarray(bk.reshape(P, 1)),
        wv_t=np.ascontiguousarray(Wv.T),
        bo=np.ascontiguousarray((bo + Wo @ bv).reshape(P, 1)).astype(
            np.float32),
    )
    import ml_dtypes
    d["vones"] = np.ones((P, P), ml_dtypes.bfloat16)
    d["wones"] = np.ones((1, HD), np.float32)
    for h in range(HEADS):
        d[f"wo_{h}"] = np.ascontiguousarray(Wo[:, HD * h:HD * (h + 1)].T)
    return d


LAST_RESULTS = None


def kernel(x, Wq, bq, Wk, bk, Wv, bv, Wo, bo):
    global LAST_RESULTS
    import os
    from concourse.bass_utils import run_bass_kernel_spmd

    x = np.asarray(x, np.float32)
    B, C, H, W = x.shape
    N = H * W
    key = ("nc", N)
    if key not in _CACHE:
        _CACHE[key] = _build_nc(N)
    nc = _CACHE[key]

    wmap = prep_weights(np.asarray(Wq, np.float32), np.asarray(bq, np.float32),
                        np.asarray(Wk, np.float32), np.asarray(bk, np.float32),
                        np.asarray(Wv, np.float32), np.asarray(bv, np.float32),
                        np.asarray(Wo, np.float32), np.asarray(bo, np.float32))

    in_maps = []
    for b in range(B):
        m = dict(wmap)
        m["x"] = np.ascontiguousarray(x[b].reshape(C, N))
        in_maps.append(m)

    tmpdir = os.environ.get("KERNEL_TMPDIR") or None
    res = run_bass_kernel_spmd(nc, in_maps, core_ids=list(range(B)),
                               tmpdir=tmpdir)
    LAST_RESULTS = res
    out = np.stack([res.results[b]["out"] for b in range(B)], axis=0)
    return out.reshape(B, C, H, W).astype(np.float32)
